# revision 25
# baseline (speedup 1.0000x reference)
"""Trainium2 Bass kernel for nn_LocalizationLoss (planar-bf16, chunk-packed).

Loss (see reference):
  p = out[:,:,0]; t = tgt[:,:,0] in {0,1}; mask = t
  bce  = -mean(t*ln(p) + (1-t)*ln(1-p))
  trick= out * t[...,None]
  CE over slot axis (dim 1) of trick[:,:,4:7] with targets tgt[:,:,4]
  Lx   = mean((trick_x - tx)^2), Ly likewise
  Lwh  = mean((t*sqrt(ow) - sqrt(tw))^2)
  loss = 5*(Lx+Ly+2*Lwh) + bce + 0.5*(1-bce) + 3*ce

Host re-layouts each core's shard into channel-planar bf16, packed
chunk-major so each chunk is ONE DMA of 128 contiguous partition lines:
  per chunk: [128, 9*Rc + 15*Gc] =
    rows planes (Rc each): p, t, tm1(=t-1), ox, oy, tx, ty, ow, tw
    group planes (Gc each): LG(i,j) 9, t_slot(i) 3, c(j) 3
Rows r=(g,i) are (group, slot); partition owns a contiguous row range.

Device per chunk computes partial sums (ACT/DVE accumulators):
  S_bce2 = sum ln((p+t-1)^2 + 1e-6)        [= 2*sum ln|p+t-1| clamped]
  S_sqxy = sum (t*ox-tx)^2 + (t*oy-ty)^2
  S_mwtw = sum (t*ow + tw)
  S_ts2  = sum 2*sqrt(t*ow*tw)  [exp(0.5*ln(W*tw + 1e-20) + ln2)]
  S_lse  = sum_j ln sum_i exp(t_i*L_ij)
  S_seli = sum_j (c_j==i) * t_i*L_ij
Host: Swh = S_mwtw - S_ts2;  ce*3B = S_lse - sum_i S_seli
      loss = 0.5 + (5*S_sqxy + 10*Swh - 0.25*S_bce2 + 3*ce*3B) / (3B)

Engines: DVE all dense-bf16 2x tensor ops + STT accumulations, ACT the
ln/exp/square chains with fused accumulation, SP (sync) one HWDGE DMA
per chunk. GpSimd/PE idle (Pool compute contends SBUF; measured).
"""

import numpy as np
import ml_dtypes

import concourse.bass as bass
import concourse.bacc as bacc
import concourse.mybir as mybir
from concourse.tile import TileContext
from concourse.bass_utils import run_bass_kernel_spmd

# Force the ACT table pass to use only natural_log_exp_and_others (it holds
# every func this kernel needs: ln/exp/square/copy). The default greedy
# per-func set choice thrashes between sets, costing ~1.3us ACT_TABLE_LOAD.
import concourse.hw_specs as _hw_specs
if not hasattr(_hw_specs, "_orig_get_activation_tables"):
    _hw_specs._orig_get_activation_tables = _hw_specs.get_activation_tables

    def _only_ln_exp_tables(module_arch):
        tabs = _hw_specs._orig_get_activation_tables(module_arch)
        return {
            name: (funcs if name == "natural_log_exp_and_others" else set())
            for name, funcs in tabs.items()
        }

    _hw_specs.get_activation_tables = _only_ln_exp_tables
    import concourse.bacc as _bacc_mod
    if hasattr(_bacc_mod, "get_activation_tables"):
        _bacc_mod.get_activation_tables = _only_ln_exp_tables

F32 = mybir.dt.float32
BF16 = mybir.dt.bfloat16
ALU = mybir.AluOpType
ACT = mybir.ActivationFunctionType
LN2 = 0.6931471805599453
EPS_BCE = 1e-6
EPS_WH = 1e-20

P = 128
N_CORES = 8
B_FULL = 1_048_576

# row-plane indices (Rc-sized each)
RP_P, RP_T, RP_TM1, RP_OX, RP_OY, RP_TX, RP_TY, RP_OW, RP_TW = range(9)
NROWP = 9
NGRPP = 15       # 9 LG + 3 t_slot + 3 c

(COL_BCE2, COL_SQXY, COL_MWTW, COL_TS2, COL_LSE,
 COL_SEL0, COL_SEL1, COL_SEL2) = range(8)
NCOL_PER_CHUNK = 8

CHUNKS_FULL = (576, 1152, 1344)        # rpp = 3072 for the full problem


def _chunk_words(R):
    return NROWP * R + NGRPP * (R // 3)


def build_kernel(nb: int, chunks) -> bass.Bass:
    rows = nb * 3
    assert rows % P == 0
    rpp = rows // P
    chunks = list(chunks)
    assert sum(chunks) == rpp, (sum(chunks), rpp)
    assert all(r % 3 == 0 for r in chunks)
    n_chunks = len(chunks)
    ncols = NCOL_PER_CHUNK * n_chunks
    total_words = sum(_chunk_words(R) for R in chunks)

    nc = bacc.Bacc()

    for val in (EPS_BCE, EPS_WH, LN2):
        ctile = nc.alloc_sbuf_tensor(f"const-f32-{val}", [128, 1], F32)
        nc.gpsimd.memset(ctile.ap(), val)
        nc.const_aps.aps[(F32, val)] = ctile.ap()
    nc.all_engine_barrier()

    data_hbm = nc.declare_dram_parameter("data", [P * total_words], BF16,
                                         isOutput=False)
    res_hbm = nc.declare_dram_parameter("res", [P, ncols], F32, isOutput=True)

    with TileContext(nc) as tc:
        with (
            tc.tile_pool(name="io", bufs=2) as io_pool,
            tc.tile_pool(name="mid", bufs=2) as mid_pool,
            tc.tile_pool(name="accp", bufs=1) as acc_pool,
        ):
            cols = acc_pool.tile([P, ncols], F32)
            probe = acc_pool.tile([P, 1], F32)
            word0 = 0
            for c, R in enumerate(chunks):
                cb = c * NCOL_PER_CHUNK
                G = R // 3
                W_CH = _chunk_words(R)
                rw = NROWP * R
                gw = NGRPP * G
                src_rows = (
                    data_hbm[word0 * P:(word0 + rw) * P]
                    .rearrange("(p x) -> p x", p=P)
                )
                src_grp = (
                    data_hbm[(word0 + rw) * P:(word0 + W_CH) * P]
                    .rearrange("(p x) -> p x", p=P)
                )
                word0 += W_CH

                iot = io_pool.tile([P, W_CH], BF16, tag="iot")
                nc.sync.dma_start(out=iot[:, 0:rw], in_=src_rows)
                nc.sync.dma_start(out=iot[:, rw:W_CH], in_=src_grp)

                def rp(k, n=1):
                    return iot[:, k * R:(k + n) * R]

                g_base = NROWP * R
                t_row = rp(RP_T)
                t_b2 = (
                    iot[:, RP_T * R:(RP_T + 1) * R]
                    .rearrange("p (one x) -> p one x", one=1)
                    .broadcast_to([P, 2, R])
                )
                lg = iot[:, g_base:g_base + 9 * G].rearrange(
                    "p (i j g) -> p i j g", i=3, j=3)
                t_slot_b = (
                    iot[:, g_base + 9 * G:g_base + 12 * G]
                    .rearrange("p (i one g) -> p i one g", i=3, one=1)
                    .broadcast_to([P, 3, 3, G])
                )
                c_jg = iot[:, g_base + 12 * G:g_base + 15 * G]

                # ---- scratch tiles ----
                q = mid_pool.tile([P, R], BF16, tag="q")
                sq = mid_pool.tile([P, R], BF16, tag="sq")
                Mxy = mid_pool.tile([P, 2 * R], BF16, tag="Mxy")
                exy = mid_pool.tile([P, 2 * R], BF16, tag="exy")
                W = mid_pool.tile([P, R], BF16, tag="W")
                m2 = mid_pool.tile([P, R], F32, tag="m2")
                s2 = mid_pool.tile([P, R], BF16, tag="s2")
                Mlog = mid_pool.tile([P, 3 * R], BF16, tag="Mlog")
                E = mid_pool.tile([P, 3 * R], BF16, tag="E")
                S = mid_pool.tile([P, R], BF16, tag="S")
                j1 = mid_pool.tile([P, R], BF16, tag="j1")
                j2 = mid_pool.tile([P, 2 * R], BF16, tag="j2")
                j3 = mid_pool.tile([P, R], BF16, tag="j3")
                j4 = mid_pool.tile([P, R], BF16, tag="j4")
                j5 = mid_pool.tile([P, R], BF16, tag="j5")

                Mxy_pl = Mxy[:, :].rearrange("p (c r) -> p c r", c=2)
                Mlog_ijg = Mlog[:, :].rearrange("p (i j g) -> p i j g", i=3, j=3)
                E_i = E[:, :].rearrange("p (i x) -> p i x", i=3)

                # ---- BCE: q' = p + (t-1) bf16; ln(q'^2 + eps) accum ----
                nc.vector.tensor_tensor(q[:, :], rp(RP_P), rp(RP_TM1), ALU.add)
                nc.scalar.activation(sq[:, :], q[:, :], ACT.Square)
                nc.scalar.activation(
                    sq[:, :], sq[:, :], ACT.Ln, bias=EPS_BCE, scale=1.0,
                    accum_out=cols[:, cb + COL_BCE2:cb + COL_BCE2 + 1],
                )
                if c == 1:
                    # perf probe: dense TS with accum (rate test, unused col)
                    nc.vector.tensor_scalar(
                        j1[:, :], sq[:, :], 1.0, None, ALU.mult, ALU.add,
                        accum_out=probe[:, 0:1],
                    )

                # ---- xy: Mxy = o_xy*t ; exy = Mxy - t_xy ; ACT sq accum ----
                nc.vector.tensor_tensor(Mxy_pl, rp(RP_OX, 2).rearrange(
                    "p (c r) -> p c r", c=2), t_b2, ALU.mult)
                nc.vector.tensor_tensor(exy[:, :], Mxy[:, :], rp(RP_TX, 2),
                                        ALU.subtract)
                nc.scalar.activation(
                    j2[:, :], exy[:, :], ACT.Square,
                    accum_out=cols[:, cb + COL_SQXY:cb + COL_SQXY + 1],
                )

                # ---- wh: W = t*ow ; mwtw = W+tw (STT accum); m2 = W*tw;
                #      s2 = exp(0.5*ln(m2+eps)+ln2) accum ----
                nc.vector.tensor_tensor(W[:, :], rp(RP_OW), t_row, ALU.mult)
                nc.vector.scalar_tensor_tensor(
                    j1[:, :], W[:, :], 0.0, rp(RP_TW), ALU.add, ALU.add,
                    accum_out=cols[:, cb + COL_MWTW:cb + COL_MWTW + 1],
                )
                nc.vector.tensor_tensor(m2[:, :], W[:, :], rp(RP_TW), ALU.mult)
                nc.scalar.activation(m2[:, :], m2[:, :], ACT.Ln,
                                     bias=EPS_WH, scale=1.0)
                nc.scalar.activation(
                    s2[:, :], m2[:, :], ACT.Exp, bias=LN2, scale=0.5,
                    accum_out=cols[:, cb + COL_TS2:cb + COL_TS2 + 1],
                )

                # ---- CE: Mlog = LG * t_slot ; E = exp ; S = sum_i ; ln ----
                nc.vector.tensor_tensor(Mlog_ijg, lg, t_slot_b, ALU.mult)
                nc.scalar.activation(E[:, :], Mlog[:, :], ACT.Exp)
                nc.vector.tensor_tensor(S[:, :], E_i[:, 0], E_i[:, 1], ALU.add)
                nc.vector.tensor_tensor(S[:, :], S[:, :], E_i[:, 2], ALU.add)
                nc.scalar.activation(
                    S[:, :], S[:, :], ACT.Ln,
                    accum_out=cols[:, cb + COL_LSE:cb + COL_LSE + 1],
                )

                # ---- CE select: (c==i)*Mlog_i STT accum ----
                for i, jt in ((0, j3), (1, j4), (2, j5)):
                    nc.vector.scalar_tensor_tensor(
                        jt[:, :], c_jg, float(i), Mlog[:, i * R:(i + 1) * R],
                        ALU.is_equal, ALU.mult,
                        accum_out=cols[:, cb + COL_SEL0 + i:cb + COL_SEL0 + i + 1],
                    )

            nc.sync.dma_start(out=res_hbm[:, :], in_=cols[:, :])

    nc.compile()
    return nc


def combine_results(res_list, n_chunks: int, b_total: int) -> np.float32:
    acc = np.zeros(NCOL_PER_CHUNK, dtype=np.float64)
    for res in res_list:
        r = np.asarray(res).astype(np.float64).reshape(P, n_chunks,
                                                       NCOL_PER_CHUNK)
        acc += r.sum(axis=(0, 1))
    s_bce = acc[COL_BCE2] * 0.5
    s_sqxy = acc[COL_SQXY]
    s_wh = acc[COL_MWTW] - acc[COL_TS2]
    s_ce = acc[COL_LSE] - (acc[COL_SEL0] + acc[COL_SEL1] + acc[COL_SEL2])
    denom = 3.0 * b_total
    loss = 0.5 + (5.0 * s_sqxy + 10.0 * s_wh - 0.5 * s_bce + 3.0 * s_ce) / denom
    return np.float32(loss)


def shard_inputs(output: np.ndarray, target: np.ndarray, chunks=None):
    """Host-side planar bf16 chunk-packed re-layout, one array per core."""
    b = output.shape[0]
    nb = b // N_CORES
    rows = nb * 3
    rpp = rows // P
    gpp = rpp // 3
    if chunks is None:
        chunks = _chunks_for(nb)
    in_maps = []
    for k in range(N_CORES):
        o = output[k * nb:(k + 1) * nb]
        t = target[k * nb:(k + 1) * nb]
        ob = o.astype(ml_dtypes.bfloat16)
        tb = t.astype(ml_dtypes.bfloat16)
        tm1 = (t[:, :, 0] - 1.0).astype(ml_dtypes.bfloat16)

        def rowplane(a):
            return a.reshape(P, rpp)
        rowp = [
            rowplane(ob[:, :, 0]), rowplane(tb[:, :, 0]), rowplane(tm1),
            rowplane(ob[:, :, 1]), rowplane(ob[:, :, 2]),
            rowplane(tb[:, :, 1]), rowplane(tb[:, :, 2]),
            rowplane(ob[:, :, 3]), rowplane(tb[:, :, 3]),
        ]                                           # 9 x [128, rpp]
        lg = ob[:, :, 4:7].reshape(P, gpp, 3, 3)    # [p, g, i, j]
        lg = lg.transpose(2, 3, 0, 1).reshape(9, P, gpp)
        ts = tb[:, :, 0].reshape(P, gpp, 3).transpose(2, 0, 1)   # [i, p, g]
        cj = tb[:, :, 4].reshape(P, gpp, 3).transpose(2, 0, 1)   # [j, p, g]
        grp = list(lg) + list(ts) + list(cj)        # 15 x [128, gpp]

        parts = []
        r0 = 0
        g0 = 0
        for R in chunks:
            G = R // 3
            rows_blk = np.ascontiguousarray(
                np.concatenate([pl[:, r0:r0 + R] for pl in rowp], axis=1))
            grp_blk = np.ascontiguousarray(
                np.concatenate([pl[:, g0:g0 + G] for pl in grp], axis=1))
            parts.append(rows_blk.reshape(-1))
            parts.append(grp_blk.reshape(-1))
            r0 += R
            g0 += G
        in_maps.append({"data": np.concatenate(parts)})
    return in_maps


_CACHED = {}


def _chunks_for(nb: int):
    rpp = nb * 3 // P
    if rpp == 3072:
        return CHUNKS_FULL
    for n in (4, 2, 1):
        if rpp % n == 0 and (rpp // n) % 3 == 0:
            return (rpp // n,) * n
    return (rpp,)


def _get_nc(nb: int):
    chunks = _chunks_for(nb)
    key = (nb, chunks)
    if key not in _CACHED:
        _CACHED[key] = (build_kernel(nb, chunks), len(chunks))
    return _CACHED[key]


def run_on_cores(output: np.ndarray, target: np.ndarray, trace: bool = False):
    b = output.shape[0]
    nb = b // N_CORES
    nc, n_chunks = _get_nc(nb)
    in_maps = shard_inputs(output, target)
    results = run_bass_kernel_spmd(
        nc, in_maps, core_ids=list(range(N_CORES)), trace=trace
    )
    res_list = [r["res"] for r in results.results]
    return res_list, n_chunks, results


def kernel(output: np.ndarray, target: np.ndarray) -> np.ndarray:
    output = np.asarray(output, dtype=np.float32)
    target = np.asarray(target, dtype=np.float32)
    b = output.shape[0]
    res_list, n_chunks, _ = run_on_cores(output, target)
    return combine_results(res_list, n_chunks=n_chunks, b_total=b)


# revision 26
# speedup vs baseline: 1.0383x; 1.0383x over previous
"""Trainium2 Bass kernel for nn_LocalizationLoss (planar-bf16, chunk-packed).

Loss (see reference):
  p = out[:,:,0]; t = tgt[:,:,0] in {0,1}; mask = t
  bce  = -mean(t*ln(p) + (1-t)*ln(1-p))
  trick= out * t[...,None]
  CE over slot axis (dim 1) of trick[:,:,4:7] with targets tgt[:,:,4]
  Lx   = mean((trick_x - tx)^2), Ly likewise
  Lwh  = mean((t*sqrt(ow) - sqrt(tw))^2)
  loss = 5*(Lx+Ly+2*Lwh) + bce + 0.5*(1-bce) + 3*ce

Host re-layouts each core's shard into channel-planar bf16, packed
chunk-major so each chunk is ONE DMA of 128 contiguous partition lines:
  per chunk: [128, 9*Rc + 15*Gc] =
    rows planes (Rc each): p, t, tm1(=t-1), ox, oy, tx, ty, ow, tw
    group planes (Gc each): LG(i,j) 9, t_slot(i) 3, c(j) 3
Rows r=(g,i) are (group, slot); partition owns a contiguous row range.

Device per chunk computes partial sums (ACT/DVE accumulators):
  S_bce2 = sum ln((p+t-1)^2 + 1e-6)        [= 2*sum ln|p+t-1| clamped]
  S_sqxy = sum (t*ox-tx)^2 + (t*oy-ty)^2
  S_mwtw = sum (t*ow + tw)
  S_ts2  = sum 2*sqrt(t*ow*tw)  [exp(0.5*ln(W*tw + 1e-20) + ln2)]
  S_lse  = sum_j ln sum_i exp(t_i*L_ij)
  S_seli = sum_j (c_j==i) * t_i*L_ij
Host: Swh = S_mwtw - S_ts2;  ce*3B = S_lse - sum_i S_seli
      loss = 0.5 + (5*S_sqxy + 10*Swh - 0.25*S_bce2 + 3*ce*3B) / (3B)

Engines: DVE all dense-bf16 2x tensor ops + STT accumulations, ACT the
ln/exp/square chains with fused accumulation, SP (sync) one HWDGE DMA
per chunk. GpSimd/PE idle (Pool compute contends SBUF; measured).
"""

import numpy as np
import ml_dtypes

import concourse.bass as bass
import concourse.bacc as bacc
import concourse.mybir as mybir
from concourse.tile import TileContext
from concourse.bass_utils import run_bass_kernel_spmd

# Force the ACT table pass to use only natural_log_exp_and_others (it holds
# every func this kernel needs: ln/exp/square/copy). The default greedy
# per-func set choice thrashes between sets, costing ~1.3us ACT_TABLE_LOAD.
import concourse.hw_specs as _hw_specs
if not hasattr(_hw_specs, "_orig_get_activation_tables"):
    _hw_specs._orig_get_activation_tables = _hw_specs.get_activation_tables

    def _only_ln_exp_tables(module_arch):
        tabs = _hw_specs._orig_get_activation_tables(module_arch)
        return {
            name: (funcs if name == "natural_log_exp_and_others" else set())
            for name, funcs in tabs.items()
        }

    _hw_specs.get_activation_tables = _only_ln_exp_tables
    import concourse.bacc as _bacc_mod
    if hasattr(_bacc_mod, "get_activation_tables"):
        _bacc_mod.get_activation_tables = _only_ln_exp_tables

F32 = mybir.dt.float32
BF16 = mybir.dt.bfloat16
ALU = mybir.AluOpType
ACT = mybir.ActivationFunctionType
LN2 = 0.6931471805599453
EPS_BCE = 1e-6
EPS_WH = 1e-20

P = 128
N_CORES = 8
B_FULL = 1_048_576

# row-plane indices (Rc-sized each)
RP_P, RP_T, RP_TM1, RP_OX, RP_OY, RP_TX, RP_TY, RP_OW, RP_TW = range(9)
NROWP = 9
NGRPP = 15       # 9 LG + 3 t_slot + 3 c

(COL_BCE2, COL_SQXY, COL_MWTW, COL_TS2, COL_LSE,
 COL_SEL0, COL_SEL1, COL_SEL2) = range(8)
NCOL_PER_CHUNK = 8

CHUNKS_FULL = (192, 576, 1152, 1152)   # rpp = 3072 for the full problem


def _chunk_words(R):
    return NROWP * R + NGRPP * (R // 3)


def build_kernel(nb: int, chunks) -> bass.Bass:
    rows = nb * 3
    assert rows % P == 0
    rpp = rows // P
    chunks = list(chunks)
    assert sum(chunks) == rpp, (sum(chunks), rpp)
    assert all(r % 3 == 0 for r in chunks)
    n_chunks = len(chunks)
    ncols = NCOL_PER_CHUNK * n_chunks
    total_words = sum(_chunk_words(R) for R in chunks)

    nc = bacc.Bacc()

    for val in (EPS_BCE, EPS_WH, LN2):
        ctile = nc.alloc_sbuf_tensor(f"const-f32-{val}", [128, 1], F32)
        nc.gpsimd.memset(ctile.ap(), val)
        nc.const_aps.aps[(F32, val)] = ctile.ap()
    nc.all_engine_barrier()

    data_hbm = nc.declare_dram_parameter("data", [P * total_words], BF16,
                                         isOutput=False)
    res_hbm = nc.declare_dram_parameter("res", [P, ncols], F32, isOutput=True)

    with TileContext(nc) as tc:
        with (
            tc.tile_pool(name="io", bufs=2) as io_pool,
            tc.tile_pool(name="mid", bufs=2) as mid_pool,
            tc.tile_pool(name="accp", bufs=1) as acc_pool,
        ):
            cols = acc_pool.tile([P, ncols], F32)
            probe = acc_pool.tile([P, 1], F32)
            word0 = 0
            for c, R in enumerate(chunks):
                cb = c * NCOL_PER_CHUNK
                G = R // 3
                W_CH = _chunk_words(R)
                rw = NROWP * R
                gw = NGRPP * G
                src_rows = (
                    data_hbm[word0 * P:(word0 + rw) * P]
                    .rearrange("(p x) -> p x", p=P)
                )
                src_grp = (
                    data_hbm[(word0 + rw) * P:(word0 + W_CH) * P]
                    .rearrange("(p x) -> p x", p=P)
                )
                word0 += W_CH

                iot = io_pool.tile([P, W_CH], BF16, tag="iot")
                nc.sync.dma_start(out=iot[:, 0:rw], in_=src_rows)
                nc.sync.dma_start(out=iot[:, rw:W_CH], in_=src_grp)

                def rp(k, n=1):
                    return iot[:, k * R:(k + n) * R]

                g_base = NROWP * R
                t_row = rp(RP_T)
                t_b2 = (
                    iot[:, RP_T * R:(RP_T + 1) * R]
                    .rearrange("p (one x) -> p one x", one=1)
                    .broadcast_to([P, 2, R])
                )
                lg = iot[:, g_base:g_base + 9 * G].rearrange(
                    "p (i j g) -> p i j g", i=3, j=3)
                t_slot_b = (
                    iot[:, g_base + 9 * G:g_base + 12 * G]
                    .rearrange("p (i one g) -> p i one g", i=3, one=1)
                    .broadcast_to([P, 3, 3, G])
                )
                c_jg = iot[:, g_base + 12 * G:g_base + 15 * G]

                # ---- scratch tiles ----
                q = mid_pool.tile([P, R], BF16, tag="q")
                sq = mid_pool.tile([P, R], BF16, tag="sq")
                Mxy = mid_pool.tile([P, 2 * R], BF16, tag="Mxy")
                exy = mid_pool.tile([P, 2 * R], BF16, tag="exy")
                W = mid_pool.tile([P, R], BF16, tag="W")
                m2 = mid_pool.tile([P, R], F32, tag="m2")
                s2 = mid_pool.tile([P, R], BF16, tag="s2")
                Mlog = mid_pool.tile([P, 3 * R], BF16, tag="Mlog")
                E = mid_pool.tile([P, 3 * R], BF16, tag="E")
                S = mid_pool.tile([P, R], BF16, tag="S")
                j1 = mid_pool.tile([P, R], BF16, tag="j1")
                j2 = mid_pool.tile([P, 2 * R], BF16, tag="j2")
                j3 = mid_pool.tile([P, R], BF16, tag="j3")
                j4 = mid_pool.tile([P, R], BF16, tag="j4")
                j5 = mid_pool.tile([P, R], BF16, tag="j5")

                Mxy_pl = Mxy[:, :].rearrange("p (c r) -> p c r", c=2)
                Mlog_ijg = Mlog[:, :].rearrange("p (i j g) -> p i j g", i=3, j=3)
                E_i = E[:, :].rearrange("p (i x) -> p i x", i=3)

                # ---- BCE: q' = p + (t-1) bf16; ln(q'^2 + eps) accum ----
                nc.vector.tensor_tensor(q[:, :], rp(RP_P), rp(RP_TM1), ALU.add)
                nc.scalar.activation(sq[:, :], q[:, :], ACT.Square)
                nc.scalar.activation(
                    sq[:, :], sq[:, :], ACT.Ln, bias=EPS_BCE, scale=1.0,
                    accum_out=cols[:, cb + COL_BCE2:cb + COL_BCE2 + 1],
                )
                if c == 1:
                    # perf probe: dense TS with accum (rate test, unused col)
                    nc.vector.tensor_scalar(
                        j1[:, :], sq[:, :], 1.0, None, ALU.mult, ALU.add,
                        accum_out=probe[:, 0:1],
                    )

                # ---- xy: Mxy = o_xy*t ; exy = Mxy - t_xy ; ACT sq accum ----
                nc.vector.tensor_tensor(Mxy_pl, rp(RP_OX, 2).rearrange(
                    "p (c r) -> p c r", c=2), t_b2, ALU.mult)
                nc.vector.tensor_tensor(exy[:, :], Mxy[:, :], rp(RP_TX, 2),
                                        ALU.subtract)
                nc.scalar.activation(
                    j2[:, :], exy[:, :], ACT.Square,
                    accum_out=cols[:, cb + COL_SQXY:cb + COL_SQXY + 1],
                )

                # ---- wh: W = t*ow ; mwtw = W+tw (STT accum); m2 = W*tw;
                #      s2 = exp(0.5*ln(m2+eps)+ln2) accum ----
                nc.vector.tensor_tensor(W[:, :], rp(RP_OW), t_row, ALU.mult)
                nc.vector.scalar_tensor_tensor(
                    j1[:, :], W[:, :], 0.0, rp(RP_TW), ALU.add, ALU.add,
                    accum_out=cols[:, cb + COL_MWTW:cb + COL_MWTW + 1],
                )
                nc.vector.tensor_tensor(m2[:, :], W[:, :], rp(RP_TW), ALU.mult)
                nc.scalar.activation(m2[:, :], m2[:, :], ACT.Ln,
                                     bias=EPS_WH, scale=1.0)
                nc.scalar.activation(
                    s2[:, :], m2[:, :], ACT.Exp, bias=LN2, scale=0.5,
                    accum_out=cols[:, cb + COL_TS2:cb + COL_TS2 + 1],
                )

                # ---- CE: Mlog = LG * t_slot ; E = exp ; S = sum_i ; ln ----
                nc.vector.tensor_tensor(Mlog_ijg, lg, t_slot_b, ALU.mult)
                nc.scalar.activation(E[:, :], Mlog[:, :], ACT.Exp)
                nc.vector.tensor_tensor(S[:, :], E_i[:, 0], E_i[:, 1], ALU.add)
                nc.vector.tensor_tensor(S[:, :], S[:, :], E_i[:, 2], ALU.add)
                nc.scalar.activation(
                    S[:, :], S[:, :], ACT.Ln,
                    accum_out=cols[:, cb + COL_LSE:cb + COL_LSE + 1],
                )

                # ---- CE select: (c==i)*Mlog_i STT accum ----
                for i, jt in ((0, j3), (1, j4), (2, j5)):
                    nc.vector.scalar_tensor_tensor(
                        jt[:, :], c_jg, float(i), Mlog[:, i * R:(i + 1) * R],
                        ALU.is_equal, ALU.mult,
                        accum_out=cols[:, cb + COL_SEL0 + i:cb + COL_SEL0 + i + 1],
                    )

            nc.sync.dma_start(out=res_hbm[:, :], in_=cols[:, :])

    nc.compile()
    return nc


def combine_results(res_list, n_chunks: int, b_total: int) -> np.float32:
    acc = np.zeros(NCOL_PER_CHUNK, dtype=np.float64)
    for res in res_list:
        r = np.asarray(res).astype(np.float64).reshape(P, n_chunks,
                                                       NCOL_PER_CHUNK)
        acc += r.sum(axis=(0, 1))
    s_bce = acc[COL_BCE2] * 0.5
    s_sqxy = acc[COL_SQXY]
    s_wh = acc[COL_MWTW] - acc[COL_TS2]
    s_ce = acc[COL_LSE] - (acc[COL_SEL0] + acc[COL_SEL1] + acc[COL_SEL2])
    denom = 3.0 * b_total
    loss = 0.5 + (5.0 * s_sqxy + 10.0 * s_wh - 0.5 * s_bce + 3.0 * s_ce) / denom
    return np.float32(loss)


def shard_inputs(output: np.ndarray, target: np.ndarray, chunks=None):
    """Host-side planar bf16 chunk-packed re-layout, one array per core."""
    b = output.shape[0]
    nb = b // N_CORES
    rows = nb * 3
    rpp = rows // P
    gpp = rpp // 3
    if chunks is None:
        chunks = _chunks_for(nb)
    in_maps = []
    for k in range(N_CORES):
        o = output[k * nb:(k + 1) * nb]
        t = target[k * nb:(k + 1) * nb]
        ob = o.astype(ml_dtypes.bfloat16)
        tb = t.astype(ml_dtypes.bfloat16)
        tm1 = (t[:, :, 0] - 1.0).astype(ml_dtypes.bfloat16)

        def rowplane(a):
            return a.reshape(P, rpp)
        rowp = [
            rowplane(ob[:, :, 0]), rowplane(tb[:, :, 0]), rowplane(tm1),
            rowplane(ob[:, :, 1]), rowplane(ob[:, :, 2]),
            rowplane(tb[:, :, 1]), rowplane(tb[:, :, 2]),
            rowplane(ob[:, :, 3]), rowplane(tb[:, :, 3]),
        ]                                           # 9 x [128, rpp]
        lg = ob[:, :, 4:7].reshape(P, gpp, 3, 3)    # [p, g, i, j]
        lg = lg.transpose(2, 3, 0, 1).reshape(9, P, gpp)
        ts = tb[:, :, 0].reshape(P, gpp, 3).transpose(2, 0, 1)   # [i, p, g]
        cj = tb[:, :, 4].reshape(P, gpp, 3).transpose(2, 0, 1)   # [j, p, g]
        grp = list(lg) + list(ts) + list(cj)        # 15 x [128, gpp]

        parts = []
        r0 = 0
        g0 = 0
        for R in chunks:
            G = R // 3
            rows_blk = np.ascontiguousarray(
                np.concatenate([pl[:, r0:r0 + R] for pl in rowp], axis=1))
            grp_blk = np.ascontiguousarray(
                np.concatenate([pl[:, g0:g0 + G] for pl in grp], axis=1))
            parts.append(rows_blk.reshape(-1))
            parts.append(grp_blk.reshape(-1))
            r0 += R
            g0 += G
        in_maps.append({"data": np.concatenate(parts)})
    return in_maps


_CACHED = {}


def _chunks_for(nb: int):
    rpp = nb * 3 // P
    if rpp == 3072:
        return CHUNKS_FULL
    for n in (4, 2, 1):
        if rpp % n == 0 and (rpp // n) % 3 == 0:
            return (rpp // n,) * n
    return (rpp,)


def _get_nc(nb: int):
    chunks = _chunks_for(nb)
    key = (nb, chunks)
    if key not in _CACHED:
        _CACHED[key] = (build_kernel(nb, chunks), len(chunks))
    return _CACHED[key]


def run_on_cores(output: np.ndarray, target: np.ndarray, trace: bool = False):
    b = output.shape[0]
    nb = b // N_CORES
    nc, n_chunks = _get_nc(nb)
    in_maps = shard_inputs(output, target)
    results = run_bass_kernel_spmd(
        nc, in_maps, core_ids=list(range(N_CORES)), trace=trace
    )
    res_list = [r["res"] for r in results.results]
    return res_list, n_chunks, results


def kernel(output: np.ndarray, target: np.ndarray) -> np.ndarray:
    output = np.asarray(output, dtype=np.float32)
    target = np.asarray(target, dtype=np.float32)
    b = output.shape[0]
    res_list, n_chunks, _ = run_on_cores(output, target)
    return combine_results(res_list, n_chunks=n_chunks, b_total=b)
